# revision 27
# baseline (speedup 1.0000x reference)
"""Multi-head attention (B=16, C=256, N=1024, H=4 heads) on 8 TRN2 NeuronCores.

Data-parallel over batch: 2 images per core, weights replicated, no
collectives. All GEMMs run in bf16 with fp32 PSUM accumulation (simulated
end-to-end rel err ~5e-4); softmax statistics, normalization and the
residual path stay fp32.

Layout strategy: everything stays "transposed" ([feature, token]) so the
whole chain — qk projection, scores, AV, out projection — needs zero
on-chip transposes:
  qkT[3C', N]  = W_proj_slices.T @ x_r          (lhsT = W_proj, rhs = x natural)
  attT[j, i]   = k @ q.T                        (lhsT = kT cols, rhs = qT)
  E            = exp(attT * scale)              (ScalarE, PSUM -> SBUF, bf16)
  outT[d, i]   = v.T @ E  (lhsT = v natural)    + ones-lhsT matmul -> denominator
  resT[c, i]   = W_out.T @ concatT + bias + x_r (exact output DRAM layout)
The softmax denominator comes from a [128,128] ones lhsT matmul over E's
j-tiles: every PSUM partition row holds s[i], i.e. already broadcast.

Scheduling notes (measured on HW):
 - DMAs ordered so the first head's weights + x land first; dummy bf16
   warmup matmuls bridge the initial DMA wait and keep the PE clock-gate
   (HAM) warm so real matmuls start at 2.4 GHz.
 - PSUM->SBUF copies ride the ScalarEngine; the DVE is kept nearly
   dedicated to the softmax drain (reciprocal_approx_fast + normalize
   muls) so AV accumulator banks recycle fast.
 - Weights/x are DMA'd as fp32 and cast to bf16 on-chip (DMA cannot
   convert dtypes).
"""
import sys

try:
    import concourse.bass as bass  # noqa: F401
except ImportError:
    sys.path.insert(0, "/opt/trn_rl_repo")

from contextlib import ExitStack

import numpy as np

import concourse.bass as bass
import concourse.mybir as mybir
import concourse.tile as tile
from concourse import bacc
from concourse.bass_utils import run_bass_kernel_spmd

F32 = mybir.dt.float32
BF16 = mybir.dt.bfloat16
EXP = mybir.ActivationFunctionType.Exp
IDENT = mybir.ActivationFunctionType.Identity

B_PER_CORE = 2   # 16 images / 8 cores
C = 256          # channels == head dim
N = 1024         # tokens (32*32)
HEADS = 4
SCALE = C ** -0.5
N_CORES = 8


def _build():
    nc = bacc.Bacc("TRN2", debug=False, num_devices=N_CORES)
    x_d = nc.declare_dram_parameter("x", [B_PER_CORE, C, N], F32, isOutput=False)
    wp_d = nc.declare_dram_parameter("W_proj", [C, 3 * HEADS * C], F32, isOutput=False)
    bp_d = nc.declare_dram_parameter("b_proj", [3 * HEADS * C], F32, isOutput=False)
    wo_d = nc.declare_dram_parameter("W_out", [HEADS * C, C], F32, isOutput=False)
    bo_d = nc.declare_dram_parameter("b_out", [C], F32, isOutput=False)
    out_d = nc.declare_dram_parameter("out", [B_PER_CORE, C, N], F32, isOutput=True)

    with tile.TileContext(nc) as tc, ExitStack() as ctx:
        pool = ctx.enter_context(tc.tile_pool(name="persist", bufs=1))
        stage_pool = ctx.enter_context(tc.tile_pool(name="stage", bufs=3))
        xr_pool = ctx.enter_context(tc.tile_pool(name="xr", bufs=2))
        xb_pool = ctx.enter_context(tc.tile_pool(name="xb", bufs=2))
        v2_pool = ctx.enter_context(tc.tile_pool(name="v2", bufs=1))
        qk_pool = ctx.enter_context(tc.tile_pool(name="qk", bufs=2))
        e_pool = ctx.enter_context(tc.tile_pool(name="e", bufs=2))
        cat_pool = ctx.enter_context(tc.tile_pool(name="cat", bufs=1))
        r_pool = ctx.enter_context(tc.tile_pool(name="r", bufs=2))
        xrb_pool = ctx.enter_context(tc.tile_pool(name="xrb", bufs=2))
        out_pool = ctx.enter_context(tc.tile_pool(name="outs", bufs=4))
        ps_work = ctx.enter_context(tc.tile_pool(name="psw", bufs=5, space="PSUM"))
        ps_acc = ctx.enter_context(tc.tile_pool(name="psa", bufs=3, space="PSUM"))

        # ---- DMAs + on-chip bf16 casts, first-needed data first ----
        xr_tiles = []
        xr = xr_pool.tile([128, 2, N], F32, tag="xr")
        for kt in range(2):
            for isl in range(2):
                nc.sync.dma_start(
                    out=xr[:, kt, isl * 512:(isl + 1) * 512],
                    in_=x_d[0, kt * 128:(kt + 1) * 128, isl * 512:(isl + 1) * 512])
        xr_tiles.append(xr)

        w_sb = pool.tile([128, 2, 3072], BF16)  # W_proj k-tiles, per-head chunks
        b_sb = None
        for h in range(HEADS):
            for kt in range(2):
                ws = stage_pool.tile([128, 768], F32, tag="wstage")
                nc.sync.dma_start(
                    out=ws[:],
                    in_=wp_d[kt * 128:(kt + 1) * 128, h * 768:(h + 1) * 768])
                nc.vector.tensor_copy(w_sb[:, kt, h * 768:(h + 1) * 768], ws[:])
            if h == 0:
                # biases: needed by the first qk PSUM->SBUF copy, not the MMs
                b_sb = pool.tile([128, 24], F32)  # b_proj, tile t
                nc.sync.dma_start(
                    out=b_sb[:], in_=bp_d[:].rearrange("(t p) -> p t", p=128))
                bo_sb = pool.tile([128, 2], F32)
                nc.sync.dma_start(out=bo_sb[:],
                                  in_=bo_d[:].rearrange("(t p) -> p t", p=128))

        wo_sb = pool.tile([128, 8, 256], BF16)  # W_out k-tiles
        for kt in range(8):
            ws = stage_pool.tile([128, 256], F32, tag="wostage")
            nc.sync.dma_start(out=ws[:], in_=wo_d[kt * 128:(kt + 1) * 128, :])
            nc.vector.tensor_copy(wo_sb[:, kt, :], ws[:])

        # second image's x: queued last, prefetched during image-0 compute
        xr = xr_pool.tile([128, 2, N], F32, tag="xr")
        for kt in range(2):
            nc.sync.dma_start(out=xr[:, kt, :],
                              in_=x_d[1, kt * 128:(kt + 1) * 128, :])
        xr_tiles.append(xr)

        # ---- small constants ----
        ones_f = pool.tile([128, 512], F32)
        nc.vector.memset(ones_f[:], 1.0)
        ones_w = pool.tile([128, 512], BF16)
        nc.vector.tensor_copy(ones_w[:], ones_f[:])
        ones_sb = ones_w[:, 0:128]

        # dummy matmuls: fill the initial DMA wait + warm the HAM clock gate
        for wi in range(20):
            warm_ps = ps_work.tile([128, 512], F32, tag="work")
            nc.tensor.matmul(out=warm_ps[:], lhsT=ones_sb, rhs=ones_w[:],
                             start=True, stop=True)

        # b_v rhs for the bias matvec (padded free dim, see total_bias below)
        zb = pool.tile([128, 8, 2], BF16)
        zscr = pool.tile([128, 16], F32)
        nc.vector.memset(zscr[:], 0.0)
        nc.vector.tensor_copy(zb[:], zscr[:].rearrange("p (a b) -> p a b", b=2))
        for kt in range(8):
            h, dt = kt // 2, kt % 2
            nc.vector.tensor_copy(zb[:, kt, 0:1], b_sb[:, h * 6 + 4 + dt:h * 6 + 5 + dt])
        total_bias = pool.tile([128, 2], F32)

        def qk_proj(xb, h):
            """q,k for head h -> [128, 4(q0 q1 k0 k1), N] bf16."""
            qk = qk_pool.tile([128, 4, N], BF16, tag="qk")
            for mt in range(4):
                cols = h * 768 + mt * 128
                ps0 = ps_work.tile([128, 512], F32, tag="work")
                ps1 = ps_work.tile([128, 512], F32, tag="work")
                ps = [ps0, ps1]
                for kt in range(2):
                    for isl in range(2):
                        nc.tensor.matmul(
                            out=ps[isl][:],
                            lhsT=w_sb[:, kt, cols:cols + 128],
                            rhs=xb[:, kt, isl * 512:(isl + 1) * 512],
                            start=(kt == 0), stop=(kt == 1))
                for isl in range(2):
                    nc.scalar.activation(qk[:, mt, isl * 512:(isl + 1) * 512],
                                         ps[isl][:], IDENT,
                                         bias=b_sb[:, h * 6 + mt:h * 6 + mt + 1])
            return qk

        def v_proj(xb, v2, hp):
            """v for heads 2hp, 2hp+1 -> v2[:, it, h*256+d] (natural layout)."""
            for it in range(8):
                ps = ps_work.tile([128, 512], F32, tag="work")
                for kt in range(2):
                    rhs = w_sb[:, kt, :].rearrange(
                        "p (h c) -> p h c", h=HEADS
                    )[:, 2 * hp:2 * hp + 2, 512:768]
                    nc.tensor.matmul(out=ps[:],
                                     lhsT=xb[:, kt, it * 128:(it + 1) * 128],
                                     rhs=rhs, start=(kt == 0), stop=(kt == 1))
                nc.scalar.copy(v2[:, it, hp * 512:(hp + 1) * 512], ps[:])

        def attT_e(qk):
            """scores attT[j, i] -> E = exp(attT * scale)."""
            e_t = e_pool.tile([128, 8, N], BF16, tag="e")
            for isl in range(2):
                for jt in range(8):
                    ps = ps_work.tile([128, 512], F32, tag="work")
                    for dt in range(2):
                        nc.tensor.matmul(
                            out=ps[:],
                            lhsT=qk[:, 2 + dt, jt * 128:(jt + 1) * 128],
                            rhs=qk[:, dt, isl * 512:(isl + 1) * 512],
                            start=(dt == 0), stop=(dt == 1))
                    nc.scalar.activation(e_t[:, jt, isl * 512:(isl + 1) * 512],
                                         ps[:], EXP, scale=SCALE)
            return e_t

        def av_isl(e_t, v2, cat, h, isl):
            """AV + denominator for one i-half; normalized into concatT."""
            o_ps0 = ps_acc.tile([128, 512], F32, tag="acc")
            o_ps1 = ps_acc.tile([128, 512], F32, tag="acc")
            s_ps = ps_acc.tile([128, 512], F32, tag="acc")
            for jt in range(8):
                e_ap = e_t[:, jt, isl * 512:(isl + 1) * 512]
                st, sp = (jt == 0), (jt == 7)
                nc.tensor.matmul(out=o_ps0[:], rhs=e_ap, start=st, stop=sp,
                                 lhsT=v2[:, jt, h * 256:h * 256 + 128])
                nc.tensor.matmul(out=o_ps1[:], rhs=e_ap, start=st, stop=sp,
                                 lhsT=v2[:, jt, h * 256 + 128:h * 256 + 256])
                nc.tensor.matmul(out=s_ps[:], rhs=e_ap, start=st, stop=sp,
                                 lhsT=ones_sb)
            r_sb = r_pool.tile([128, 512], F32, tag="r")
            nc.vector.reciprocal_approx_fast(r_sb[:], s_ps[:])
            nc.vector.tensor_mul(
                cat[:, 2 * h, isl * 512:(isl + 1) * 512], o_ps0[:], r_sb[:])
            nc.vector.tensor_mul(
                cat[:, 2 * h + 1, isl * 512:(isl + 1) * 512], o_ps1[:], r_sb[:])

        for b in range(B_PER_CORE):
            xr = xr_tiles[b]
            xb = xb_pool.tile([128, 2, N], BF16, tag="xb")
            nc.vector.tensor_copy(xb[:], xr[:])
            v2 = v2_pool.tile([128, 8, 1024], BF16, tag="v2")
            cat = cat_pool.tile([128, 8, N], BF16, tag="cat")

            qk = qk_proj(xb, 0)
            v_proj(xb, v2, 0)
            e_t = attT_e(qk)
            av_isl(e_t, v2, cat, 0, 0)
            av_isl(e_t, v2, cat, 0, 1)
            qk = qk_proj(xb, 1)
            e_t = attT_e(qk)
            av_isl(e_t, v2, cat, 1, 0)
            av_isl(e_t, v2, cat, 1, 1)
            qk = qk_proj(xb, 2)
            v_proj(xb, v2, 1)
            e_t = attT_e(qk)
            av_isl(e_t, v2, cat, 2, 0)
            av_isl(e_t, v2, cat, 2, 1)
            qk = qk_proj(xb, 3)
            e_t = attT_e(qk)
            av_isl(e_t, v2, cat, 3, 0)
            av_isl(e_t, v2, cat, 3, 1)

            if b == 0:
                # b_v folds through softmax (weights sum to 1) and W_out:
                # total_bias[c] = b_out[c] + sum_hd b_v[hd] * W_out[hd, c].
                # Deferred here so it doesn't stall the PE on the W_out DMA.
                for ct in range(2):
                    bias_ps = ps_work.tile([128, 2], F32, tag="work")
                    for kt in range(8):
                        nc.tensor.matmul(out=bias_ps[:],
                                         lhsT=wo_sb[:, kt, ct * 128:(ct + 1) * 128],
                                         rhs=zb[:, kt, :],
                                         start=(kt == 0), stop=(kt == 7))
                    nc.vector.tensor_add(total_bias[:, ct:ct + 1], bias_ps[:, 0:1],
                                         bo_sb[:, ct:ct + 1])

            # residual + bias, broadcast along tokens: xrb = x_r + total_bias
            xrb = xrb_pool.tile([128, 2, N], F32, tag="xrb")
            for ct in range(2):
                nc.scalar.activation(xrb[:, ct, :], xr[:, ct, :],
                                     IDENT, bias=total_bias[:, ct:ct + 1])

            # ---- out projection + residual, already in output layout ----
            for ct in range(2):
                for isl in range(2):
                    res_ps = ps_work.tile([128, 512], F32, tag="work")
                    for kt in range(8):
                        nc.tensor.matmul(
                            out=res_ps[:],
                            lhsT=wo_sb[:, kt, ct * 128:(ct + 1) * 128],
                            rhs=cat[:, kt, isl * 512:(isl + 1) * 512],
                            start=(kt == 0), stop=(kt == 7))
                    o_sb = out_pool.tile([128, 512], F32, tag="o_sb")
                    nc.vector.tensor_add(o_sb[:], res_ps[:],
                                         xrb[:, ct, isl * 512:(isl + 1) * 512])
                    nc.sync.dma_start(
                        out=out_d[b, ct * 128:(ct + 1) * 128,
                                  isl * 512:(isl + 1) * 512],
                        in_=o_sb[:])

    nc.compile()
    return nc


_NC = None


def kernel(x, W_proj, b_proj, W_out, b_out):
    global _NC
    if _NC is None:
        _NC = _build()
    x = np.ascontiguousarray(x, dtype=np.float32).reshape(16, C, N)
    in_maps = [
        {
            "x": x[i * B_PER_CORE:(i + 1) * B_PER_CORE],
            "W_proj": np.ascontiguousarray(W_proj, dtype=np.float32),
            "b_proj": np.ascontiguousarray(b_proj, dtype=np.float32),
            "W_out": np.ascontiguousarray(W_out, dtype=np.float32),
            "b_out": np.ascontiguousarray(b_out, dtype=np.float32),
        }
        for i in range(N_CORES)
    ]
    res = run_bass_kernel_spmd(_NC, in_maps, core_ids=list(range(N_CORES)))
    out = np.concatenate([res.results[i]["out"] for i in range(N_CORES)], axis=0)
    return out.reshape(16, C, 32, 32)
